# revision 23
# baseline (speedup 1.0000x reference)
"""Trainium2 Bass kernel for nn_ConceptLayer (B=2, S=512, E=256), 8 NeuronCores.

Math:
  s[b,i,:] = sum_{j<i} x[b,j,:] / (i-j)^2            (prefix matmul, W constant)
  y[b,i,c] = sum_{a,p} x[b,i,a] * s[b,i,p] * C[c,a,p]
  out      = LayerNorm(y + x) * gamma + beta          (eps=1e-3)

Sharding: the contraction axis `a` is split 8 ways (32 values per core) so each
core streams 1/8 of the 64MB concept_map exactly once.  Each core computes a
partial y over all 1024 tokens; a ReduceScatter(add) gives core k the summed
tokens [128k, 128k+128); each core then applies residual + LayerNorm to its
slice and the host concatenates the 8 slices.

Device algorithm per core (mode "f32"):
  phase 1: SmT[p,t] = s^T via PE (x as stationary, W^T moving), f32
  phase 2: for each pair of a's: Z = SmT^T @ [C_a1|C_a2]  (PSUM, N=512)
           y_acc[t] += x[:,a] * Z_a   (DVE scalar_tensor_tensor, fused)
  phase 3: ReduceScatter + residual + LayerNorm

Mode "bf16" (faster): fold x[:,a] into the stationary instead:
  scaled_a[p,t] = SmT[p,t]*x[t,a] built as bf16 = TT(SmT_bf16, bcast(xT[a]))
  (partition_broadcast on GPSIMD, tensor_tensor on DVE in 2x bf16 mode);
  PSUM then accumulates over ALL a's: psum_y[t] += scaled_a^T @ C_a  and the
  per-a vector work drops to one 2x-mode multiply; C is bf16 (halves DMA).
"""
import os
import numpy as np

import concourse.bass as bass
import concourse.mybir as mybir
import concourse.tile as tile
from concourse.bass_utils import run_bass_kernel_spmd

# ----------------------------------------------------------------------------
# constants (hardcoded per problem spec)
B, S, E = 2, 512, 256
T = B * S                      # 1024 tokens
NCORES = 8
ASH = E // NCORES              # 32 contraction-a values per core
TCH = T // 128                 # 8 token chunks
LN_EPS = 1e-3

F32 = mybir.dt.float32
BF16 = mybir.dt.bfloat16
MULT = mybir.AluOpType.mult
ADD = mybir.AluOpType.add
AF = mybir.ActivationFunctionType

MODE = os.environ.get("KMODE", "f32")   # "f32" | "bf16"

LAST_RESULTS = None            # BassKernelResults of the last run (for test.py)

_NC_CACHE = {}


def _install_ntff_hook():
    """antenv.axon_hooks is absent in this image; recreate it so
    run_bass_kernel_spmd(trace=True) can drive NTFF profiling via the
    libaxon_pjrt.so C ABI (same recipe as trn_agent_boot)."""
    import sys, types, ctypes, contextlib  # noqa: E401

    if "antenv.axon_hooks" in sys.modules:
        return
    so_path = "/opt/axon/libaxon_pjrt.so"
    try:
        lib = ctypes.CDLL(so_path)
    except OSError:
        return
    if not hasattr(lib, "axon_start_nrt_profile"):
        return
    lib.axon_start_nrt_profile.argtypes = [
        ctypes.POINTER(ctypes.c_int64), ctypes.c_size_t]
    lib.axon_start_nrt_profile.restype = ctypes.c_int64
    lib.axon_stop_nrt_profile.argtypes = [ctypes.c_char_p]
    lib.axon_stop_nrt_profile.restype = ctypes.c_int64

    @contextlib.contextmanager
    def _hook(output_dir, device_ids):
        import jax
        jax.devices()
        if device_ids:
            ids = (ctypes.c_int64 * len(device_ids))(*device_ids)
            rc = lib.axon_start_nrt_profile(ids, len(device_ids))
        else:
            rc = lib.axon_start_nrt_profile(None, 0)
        if rc != 0:
            raise RuntimeError(f"axon_start_nrt_profile rc={rc}")
        try:
            yield
        finally:
            n = lib.axon_stop_nrt_profile(str(output_dir).encode())
            print(f"profile: {n} file(s) written to {output_dir}")

    mod = types.ModuleType("antenv.axon_hooks")
    mod.get_axon_ntff_profile_hook = lambda: _hook
    mod.set_axon_ntff_profile_hook = lambda h: None
    sys.modules["antenv.axon_hooks"] = mod


_install_ntff_hook()


def _split_excess_waits(nc):
    """walrus CoreV3 codegen allows only one sync-wait on Drain instructions;
    Tile's tail drain aggregates one wait per outstanding semaphore.  Move the
    excess onto NOPs inserted just before the offender (same engine)."""
    for fn in nc.m.functions:
        for bb in fn.blocks:
            insts = bb.instructions
            i = 0
            while i < len(insts):
                inst = insts[i]
                si = inst.sync_info
                max_waits = 1
                if si is not None and si.on_wait and len(si.on_wait) > max_waits:
                    waits = list(si.on_wait)
                    si.on_wait = waits[:max_waits]
                    extra = waits[max_waits:]
                    new_nops = []
                    for j in range(0, len(extra), max_waits):
                        nop = nc.engines[inst.engine].nop(nofuse=True).ins
                        nop.sync_info = mybir.SyncInfo(
                            on_wait=extra[j : j + max_waits], on_update=[]
                        )
                        new_nops.append(nop)
                    for nop in new_nops:
                        for fb in fn.blocks:
                            if nop in fb.instructions:
                                fb.instructions.remove(nop)
                    idx = insts.index(inst)
                    for k, nop in enumerate(new_nops):
                        insts.insert(idx + k, nop)
                    i = insts.index(inst)
                i += 1


def _build_nc(mode, cc="rs"):
    debug_partial = mode.endswith("dbg")
    mode = mode.replace("dbg", "")
    nc = bass.Bass("TRN2", target_bir_lowering=False, debug=False,
                   num_devices=NCORES)

    cdt = F32 if mode == "f32" else BF16

    xin = nc.dram_tensor("xin", [T, E], cdt, kind="ExternalInput")
    xa = (nc.dram_tensor("xa", [T, ASH], F32, kind="ExternalInput")
          if mode == "f32" else None)
    wt = nc.dram_tensor("wt", [S, S], cdt, kind="ExternalInput")
    cs = nc.dram_tensor("cs", [ASH, E, E], cdt, kind="ExternalInput")
    xres = nc.dram_tensor("xres", [128, E], F32, kind="ExternalInput")
    gw = nc.dram_tensor("gw", [128, E], F32, kind="ExternalInput")
    bw = nc.dram_tensor("bw", [128, E], F32, kind="ExternalInput")
    if mode == "bf16":
        # xt rows: this core's 32 columns of x, transposed: (ASH, T) bf16
        xt = nc.dram_tensor("xt", [ASH, T], BF16, kind="ExternalInput")
    yout = nc.dram_tensor("yout", [128, E], F32, kind="ExternalOutput")

    ccin = nc.dram_tensor("ccin", [T, E], F32)
    dbg = (nc.dram_tensor("dbg", [T, E], F32, kind="ExternalOutput")
           if debug_partial else None)
    # ReduceScatter / AllToAll require Local (non-Shared) outputs
    ccout = nc.dram_tensor("ccout", [128, E], F32)
    a2aout = nc.dram_tensor("a2aout", [T, E], F32) if cc == "a2a" else None

    with tile.TileContext(nc) as tc:
        import contextlib
        with contextlib.ExitStack() as ctx:
            consts = ctx.enter_context(tc.tile_pool(name="consts", bufs=1))
            cpool = ctx.enter_context(tc.tile_pool(name="cpool", bufs=4))
            scld = ctx.enter_context(tc.tile_pool(name="scld", bufs=3))
            small = ctx.enter_context(tc.tile_pool(name="small", bufs=2))


            # ---------------- phase 0: load constants -----------------------
            # merged loads: one DMA each (HWDGE issue is ~0.6us per dma_start
            # and serializes per queue, so fewer+bigger is better)
            x_all = consts.tile([128, TCH, E], cdt, tag="x_all")
            nc.sync.dma_start(
                out=x_all, in_=xin.ap().rearrange("(tc p) c -> p tc c", p=128))
            x_sb = [x_all[:, t, :] for t in range(TCH)]
            wt_all = consts.tile([128, S // 128, S], cdt, tag="wt_all")
            nc.scalar.dma_start(
                out=wt_all, in_=wt.ap().rearrange("(j p) i -> p j i", p=128))
            wt_sb = [wt_all[:, j, :] for j in range(S // 128)]
            xa_sb = []
            if mode == "f32":
                for t in range(TCH):
                    a_t = consts.tile([128, ASH], F32, tag=f"xa{t}")
                    nc.sync.dma_start(out=a_t, in_=xa.ap()[t * 128:(t + 1) * 128, :])
                    xa_sb.append(a_t)
            # xres/gamma/beta are only needed for the LN tail; load late so
            # they don't delay phase 1.
            if mode == "bf16":
                # all 32 broadcast rows in one 8MB DMA on the scalar ring
                # (in AP: partition step 0 replicates; free dims (a, t))
                xt_ap = xt.ap()
                bc_all = consts.tile([128, ASH, T], BF16, tag="bc_all")
                bc_src = bass.AP(
                    tensor=xt_ap.tensor,
                    offset=xt_ap.offset,
                    ap=[[0, 128], [T, ASH], [1, T]],
                )
                nc.scalar.dma_start(out=bc_all, in_=bc_src)

            # ---------------- phase 0.5: PE warmup ---------------------------
            # ~24 dependency-free matmuls keep the PE busy from t~1us so the
            # HAM clock gate reaches 2.4GHz before real work arrives, and all
            # 8 cores start phase 2 in lockstep (less collective skew).
            wup_in = consts.tile([128, S], BF16, tag="wup_in")
            nc.gpsimd.memset(wup_in, 0.0)
            with tc.tile_pool(name="ps_w", bufs=1, space="PSUM") as ps_w:
                wps = ps_w.tile([128, S], F32, tag="wps")
                for _ in range(24):
                    nc.tensor.matmul(wps, lhsT=wup_in[:, :128], rhs=wup_in,
                                     start=True, stop=True)

            # ---------------- phase 1: SmT = (W @ x)^T ----------------------
            # SmT[d, i] per batch: lhsT = x[b] chunk (j,d), rhs = W^T (j,i)
            sdt = F32 if mode == "f32" else BF16
            smT = []
            for d in range(E // 128):
                s_t = consts.tile([128, T], sdt, tag=f"smT{d}")
                smT.append(s_t)
            with tc.tile_pool(name="ps_s", bufs=2, space="PSUM") as ps_s:
                for b in range(B):
                    for d in range(E // 128):
                        ps = ps_s.tile([128, S], F32, tag="ps_s")
                        for j in range(S // 128):
                            nc.tensor.matmul(
                                ps,
                                lhsT=x_sb[b * 4 + j][:, d * 128:(d + 1) * 128],
                                rhs=wt_sb[j],
                                start=(j == 0),
                                stop=(j == S // 128 - 1),
                            )
                        nc.scalar.copy(smT[d][:, b * S:(b + 1) * S], ps)
            if mode == "f32":
                ps_y = ctx.enter_context(
                    tc.tile_pool(name="ps_y", bufs=4, space="PSUM"))
            else:
                # 8 full banks, one per t-chunk accumulator (PSUM start=True
                # clears a whole bank, so accumulators must not share banks)
                ps_y = ctx.enter_context(
                    tc.tile_pool(name="ps_y", bufs=1, space="PSUM"))

            # ---------------- phase 2: big contraction ----------------------
            if mode == "f32":
                # y_acc[t] starts at 0
                y_acc = []
                for t in range(TCH):
                    ya = consts.tile([128, E], F32, tag=f"yacc{t}")
                    nc.gpsimd.memset(ya, 0.0)
                    y_acc.append(ya)
                for api in range(ASH // 2):        # a-pairs
                    ct = []
                    for p in range(E // 128):
                        c_t = cpool.tile([128, 2, E], F32, tag=f"ct{p}")
                        src = cs.ap()[2 * api:2 * api + 2,
                                      p * 128:(p + 1) * 128, :]
                        nc.sync.dma_start(
                            out=c_t, in_=src.rearrange("a p c -> p a c"))
                        ct.append(c_t)
                    for t in range(TCH):
                        ps = ps_y.tile([128, 2 * E], F32, tag="ps_y")
                        for p in range(E // 128):
                            nc.tensor.matmul(
                                ps,
                                lhsT=smT[p][:, t * 128:(t + 1) * 128],
                                rhs=ct[p].rearrange("p a c -> p (a c)"),
                                start=(p == 0),
                                stop=(p == E // 128 - 1),
                            )
                        for ai in range(2):
                            a = 2 * api + ai
                            nc.vector.scalar_tensor_tensor(
                                out=y_acc[t],
                                in0=ps[:, ai * E:(ai + 1) * E],
                                scalar=xa_sb[t][:, a:a + 1],
                                in1=y_acc[t],
                                op0=MULT,
                                op1=ADD,
                            )
            else:
                # bf16 fold: psum accumulates over every (a, p)
                # one full PSUM bank per t-chunk (start=True clears the whole
                # bank, so banks must not be shared between accumulators)
                y_ps = []
                for t in range(TCH):
                    y_ps.append(ps_y.tile([128, E], F32, tag=f"ypsum{t}",
                                          name=f"ypsum{t}"))
                for a in range(ASH):
                    bc = bc_all[:, a, :]
                    sc = []
                    for p in range(E // 128):
                        s_t = scld.tile([128, T], BF16, tag=f"sc{p}")
                        nc.vector.tensor_tensor(
                            out=s_t, in0=smT[p], in1=bc, op=MULT)
                        sc.append(s_t)
                    c_all = cpool.tile([128, E // 128, E], BF16, tag="c_all")
                    nc.sync.dma_start(
                        out=c_all,
                        in_=cs.ap()[a].rearrange("(pc p) c -> p pc c", p=128))
                    for t in range(TCH):
                        for p in range(E // 128):
                            nc.tensor.matmul(
                                y_ps[t],
                                lhsT=sc[p][:, t * 128:(t + 1) * 128],
                                rhs=c_all[:, p, :],
                                start=(a == 0 and p == 0),
                                stop=(a == ASH - 1 and p == E // 128 - 1),
                            )
                y_all = consts.tile([128, TCH, E], F32, tag="y_all")
                for t in range(TCH):
                    nc.scalar.copy(y_all[:, t, :], y_ps[t])
                y_acc = None

            # deferred constant loads (needed only from here on)
            xres_sb = consts.tile([128, E], F32, tag="xres")
            nc.sync.dma_start(out=xres_sb, in_=xres.ap())
            gw_sb = consts.tile([128, E], F32, tag="gw")
            nc.sync.dma_start(out=gw_sb, in_=gw.ap())
            bw_sb = consts.tile([128, E], F32, tag="bw")
            nc.sync.dma_start(out=bw_sb, in_=bw.ap())

            # ---------------- phase 3: reduce + LN --------------------------
            if y_acc is None:
                ccin_v = ccin.ap().rearrange("(tc p) c -> p tc c", p=128)
                nc.sync.dma_start(out=ccin_v, in_=y_all)
                if dbg is not None:
                    nc.sync.dma_start(
                        out=dbg.ap().rearrange("(tc p) c -> p tc c", p=128),
                        in_=y_all)
            else:
                for t in range(TCH):
                    nc.sync.dma_start(
                        out=ccin.ap()[t * 128:(t + 1) * 128, :], in_=y_acc[t])
                    if dbg is not None:
                        nc.sync.dma_start(
                            out=dbg.ap()[t * 128:(t + 1) * 128, :], in_=y_acc[t])
            if cc == "rs":
                nc.gpsimd.collective_compute(
                    "ReduceScatter",
                    ADD,
                    replica_groups=[list(range(NCORES))],
                    ins=[ccin.ap()],
                    outs=[ccout.ap()],
                )
                yred = small.tile([128, E], F32, tag="yred")
                nc.sync.dma_start(out=yred, in_=ccout.ap())
            else:
                # AllToAll: block j of ccin (= partial y for tokens of core j)
                # lands at position <us> in core j's output; each core then
                # reduces the 8 received partials locally on DVE.
                nc.gpsimd.collective_compute(
                    "AllToAll",
                    mybir.AluOpType.bypass,
                    replica_groups=[list(range(NCORES))],
                    ins=[ccin.ap()],
                    outs=[a2aout.ap()],
                )
                g = small.tile([128, NCORES, E], F32, tag="gbuf")
                nc.sync.dma_start(
                    out=g,
                    in_=a2aout.ap().rearrange("(s p) c -> p s c", p=128))
                g4 = small.tile([128, 4, E], F32, tag="g4")
                nc.vector.tensor_tensor(
                    out=g4, in0=g[:, 0:4, :], in1=g[:, 4:8, :], op=ADD)
                g2 = small.tile([128, 2, E], F32, tag="g2")
                nc.vector.tensor_tensor(
                    out=g2, in0=g4[:, 0:2, :], in1=g4[:, 2:4, :], op=ADD)
                yred = small.tile([128, E], F32, tag="yred")
                nc.vector.tensor_tensor(
                    out=yred, in0=g2[:, 0, :], in1=g2[:, 1, :], op=ADD)
            nc.vector.tensor_tensor(out=yred, in0=yred, in1=xres_sb, op=ADD)
            stats = small.tile([128, 6], F32, tag="stats")
            nc.vector.bn_stats(out=stats, in_=yred)
            mv = small.tile([128, 2], F32, tag="mv")
            nc.vector.bn_aggr(out=mv, in_=stats)
            # rstd = 1/sqrt(var + eps)
            eps_t = small.tile([128, 1], F32, tag="eps")
            nc.vector.memset(eps_t, LN_EPS)
            std = small.tile([128, 1], F32, tag="std")
            nc.scalar.activation(out=std, in_=mv[:, 1:2], func=AF.Sqrt,
                                 bias=eps_t)
            rstd = small.tile([128, 1], F32, tag="rstd")
            nc.vector.reciprocal(out=rstd, in_=std)
            cent = small.tile([128, E], F32, tag="cent")
            nc.vector.tensor_scalar(
                out=cent, in0=yred, scalar1=mv[:, 0:1], scalar2=None,
                op0=mybir.AluOpType.subtract)
            tmp = small.tile([128, E], F32, tag="tmp")
            nc.vector.scalar_tensor_tensor(
                out=tmp, in0=cent, scalar=rstd, in1=gw_sb, op0=MULT, op1=MULT)
            yfin = small.tile([128, E], F32, tag="yfin")
            nc.vector.tensor_tensor(out=yfin, in0=tmp, in1=bw_sb, op=ADD)
            nc.sync.dma_start(out=yout.ap(), in_=yfin)

    _split_excess_waits(nc)
    return nc


def _get_nc(mode, cc="rs"):
    key = (mode, cc)
    if key not in _NC_CACHE:
        _NC_CACHE[key] = _build_nc(mode, cc)
    return _NC_CACHE[key]


def _prefix_wt():
    idx = np.arange(S)
    diff = idx[:, None] - idx[None, :]          # i - j
    W = np.where(diff > 0, 1.0 / np.square(np.maximum(diff, 1)), 0.0)
    return np.ascontiguousarray(W.T.astype(np.float32))   # WT[j, i] = W[i, j]


def kernel(x, concept_map, gamma, beta, mode=None, trace=False):
    global LAST_RESULTS
    mode = mode or MODE
    base_mode = mode.replace("dbg", "")
    xf = np.ascontiguousarray(np.asarray(x, dtype=np.float32).reshape(T, E))
    cmap = np.asarray(concept_map, dtype=np.float32)
    gammaf = np.asarray(gamma, dtype=np.float32)
    betaf = np.asarray(beta, dtype=np.float32)

    wt_np = _prefix_wt()
    # C_perm[a, p, c] = concept_map[c, a, p]
    import ml_dtypes  # noqa: F401  (registers bfloat16 with numpy)
    cdt = np.float32 if base_mode == "f32" else ml_dtypes.bfloat16
    cperm = np.ascontiguousarray(np.transpose(cmap, (1, 2, 0)))
    gb = np.ascontiguousarray(np.broadcast_to(gammaf, (128, E))).astype(np.float32)
    bb = np.ascontiguousarray(np.broadcast_to(betaf, (128, E))).astype(np.float32)

    in_maps = []
    for c in range(NCORES):
        a0 = c * ASH
        im = {
            "xin": xf.astype(cdt),
            "wt": wt_np.astype(cdt),
            "cs": np.ascontiguousarray(cperm[a0:a0 + ASH]).astype(cdt),
            "xres": np.ascontiguousarray(xf[c * 128:(c + 1) * 128, :]),
            "gw": gb,
            "bw": bb,
        }
        if base_mode == "f32":
            im["xa"] = np.ascontiguousarray(xf[:, a0:a0 + ASH])
        if base_mode == "bf16":
            im["xt"] = np.ascontiguousarray(xf.T[a0:a0 + ASH, :]).astype(
                ml_dtypes.bfloat16)
        in_maps.append(im)

    cc = os.environ.get("KCC", "a2a")
    nc = _get_nc(mode, cc)
    res = None
    for attempt in range(4):
        try:
            res = run_bass_kernel_spmd(nc, in_maps, list(range(NCORES)),
                                       trace=trace)
            break
        except Exception:
            # transient NRT_EXEC_UNIT_UNRECOVERABLE happens occasionally on
            # the first dispatch after a fresh compile; back off and retry
            if attempt == 3:
                raise
            import time
            time.sleep(10 * (attempt + 1))
    LAST_RESULTS = res
    y = np.concatenate([res.results[c]["yout"] for c in range(NCORES)], axis=0)
    return np.ascontiguousarray(y.reshape(B, S, E).astype(np.float32))
